# revision 4
# baseline (speedup 1.0000x reference)
"""LoRA-MLP kernel for 8x TRN2 NeuronCores (SPMD data-parallel over batch).

Math (per batch b):
    z1 = (x @ v) / IN            [F, R]
    z  = (z1 @ u.T) / R          [F, OUT]
    y  = gelu(x @ W.T + fc_bias + z + b)

Device formulation (per core, 4 batches), output-channel-stationary:
    z1r[r, f]  = sum_kk vs8[kk].T @ xt8[kk]     (fp8e4m3 DoubleRow, K=256/mm)
    z1_sb      = z1r / (IN*R)                   (ScalarE scale, -> bf16)
    psum[o, f] = sum_k wT[k][:, o].T @ xT[k][:, f]   (bf16, 8 K-tiles of 128)
               + uT[:, o].T @ z1_sb                   (bf16, K=16 LoRA)
    yT[o, f]   = gelu(psum + biasvec[o])   (ScalarE per-partition bias, bf16)

With o on the PSUM partition dim, fc_bias + b is a per-partition scalar, so
the ScalarE activation applies it for free (no K=1 bias matmuls), and the
store is bf16 (half the store traffic).  Host un-transposes yT.
z1 runs as fp8 DoubleRow (measured 351ns vs 2x265ns bf16 for K=1024xF=512);
x and v are quantized to e4m3 for it, which only touches the LoRA term
(~0.7% of the output's magnitude), keeping overall rel err ~2e-3.

Main matmul operands bf16; fp32 accumulation in PSUM.  `reps` unrolls
whole per-core passes; `loop` adds a tc.For_i hardware loop around them
(bench-only: dispatch noise amortizes over L*reps passes).
"""

import sys

for _p in ("/opt/trn_rl_repo", "/opt/pypackages"):
    if _p not in sys.path:
        sys.path.append(_p)

import numpy as np
import ml_dtypes

B, F, IN, OUT, R = 32, 512, 1024, 1024, 16
NCORES = 8
BPC = B // NCORES  # batches per core = 4
KT = IN // 128  # 8 K-tiles (bf16 main)
KT2 = IN // 256  # 4 DoubleRow K-tiles (fp8 z1)
OT = OUT // 128  # 8 output-channel tiles
BF16 = ml_dtypes.bfloat16
E4M3 = ml_dtypes.float8_e4m3

_COMPILED = {}


def _build_nc(reps=1, loop=None):
    import contextlib

    import concourse.tile as tile
    from concourse import bacc, mybir

    # Bacc (not raw Bass): its compile() runs generate_event_semaphores,
    # which splits multi-sem waits — walrus codegen allows only one sync
    # wait per instruction.
    nc = bacc.Bacc(None)
    bf = mybir.dt.bfloat16
    f32 = mybir.dt.float32
    e4 = mybir.dt.float8e4
    DR = mybir.MatmulPerfMode.DoubleRow

    xt = nc.declare_dram_parameter("xt", [BPC, 128, KT, F], bf, isOutput=False)
    wt = nc.declare_dram_parameter("wt", [128, KT, OUT], bf, isOutput=False)
    xt8 = nc.declare_dram_parameter("xt8", [BPC, 128, KT2, 2, F], e4, isOutput=False)
    vs8 = nc.declare_dram_parameter("vs8", [BPC, 128, KT2, 2, R], e4, isOutput=False)
    ut = nc.declare_dram_parameter("ut", [BPC, R, OUT], bf, isOutput=False)
    biasv = nc.declare_dram_parameter("biasv", [128, BPC * OT], f32, isOutput=False)
    y = nc.declare_dram_parameter("y", [BPC, OT, 128, F], bf, isOutput=True)

    GELU = mybir.ActivationFunctionType.Gelu
    ZSCALE = 1.0 / float(IN * R)

    with tile.TileContext(nc) as tc:
        with (
            tc.tile_pool(name="const", bufs=1) as const_pool,
            tc.tile_pool(name="xin", bufs=BPC) as xin_pool,
            tc.tile_pool(name="small", bufs=BPC) as small_pool,
            tc.tile_pool(name="zsb", bufs=2) as zsb_pool,
            tc.tile_pool(name="out", bufs=8) as out_pool,
            tc.tile_pool(name="psum", bufs=6, space="PSUM") as psum_pool,
            tc.tile_pool(name="zpsum", bufs=2, space="PSUM") as zpsum_pool,
        ):
            wt_sb = const_pool.tile([128, KT, OUT], bf)
            nc.sync.dma_start(out=wt_sb[:], in_=wt[:])
            bias_sb = const_pool.tile([128, BPC * OT], f32)
            nc.sync.dma_start(out=bias_sb[:], in_=biasv[:])

            ctx = tc.For_i(0, loop) if loop is not None else contextlib.nullcontext()
            with ctx:
                for _ in range(reps):
                    for b in range(BPC):
                        xt_sb = xin_pool.tile([128, KT, F], bf, tag="xt")
                        nc.sync.dma_start(out=xt_sb[:], in_=xt[b])
                        xt8_sb = xin_pool.tile([128, KT2, 2, F], e4, tag="xt8")
                        nc.sync.dma_start(out=xt8_sb[:], in_=xt8[b])
                        vs8_sb = small_pool.tile([128, KT2, 2, R], e4, tag="vs")
                        nc.sync.dma_start(out=vs8_sb[:], in_=vs8[b])
                        ut_sb = small_pool.tile([R, OUT], bf, tag="ut")
                        nc.sync.dma_start(out=ut_sb[:], in_=ut[b])

                        # Stage 1: z1r[r, f] = sum x8.v8 (unscaled), fp8 DR
                        z1_ps = zpsum_pool.tile([R, F], f32, tag="z1ps")
                        for kk in range(KT2):
                            nc.tensor.matmul(
                                z1_ps[:],
                                lhsT=vs8_sb[:, kk],
                                rhs=xt8_sb[:, kk],
                                start=(kk == 0),
                                stop=(kk == KT2 - 1),
                                perf_mode=DR,
                            )
                        z1_sb = zsb_pool.tile([R, F], bf, tag="z1")
                        nc.scalar.mul(z1_sb[:], z1_ps[:], ZSCALE)

                        # Stage 2: per o-tile, main matmul + LoRA in PSUM,
                        # then fused bias+gelu on ScalarE straight to bf16.
                        for ot in range(OT):
                            osl = slice(ot * 128, (ot + 1) * 128)
                            ps = psum_pool.tile([128, F], f32, tag="ps")
                            for k in range(KT):
                                nc.tensor.matmul(
                                    ps[:],
                                    lhsT=wt_sb[:, k, osl],
                                    rhs=xt_sb[:, k, :],
                                    start=(k == 0),
                                    stop=False,
                                )
                            nc.tensor.matmul(
                                ps[:], lhsT=ut_sb[:, osl], rhs=z1_sb[:],
                                start=False, stop=True,
                            )
                            o_sb = out_pool.tile([128, F], bf, tag="o")
                            bidx = b * OT + ot
                            nc.scalar.activation(
                                o_sb[:], ps[:], GELU,
                                bias=bias_sb[:, bidx : bidx + 1],
                            )
                            nc.sync.dma_start(out=y[b, ot], in_=o_sb[:])
    nc.finalize()
    return nc


def _shard_inputs(x, u, v, b, W, fc_bias):
    """Build per-core device input dicts (host-side layout + casts)."""
    # xt[c][bb, p, k, f] = x[4c+bb, f, 128k+p]
    xr = x.reshape(B, F, KT, 128).transpose(0, 3, 2, 1)  # [B,128,KT,F]
    xt = np.ascontiguousarray(xr).astype(BF16)
    # xt8: fp8 copy in DoubleRow packing: [B, 128, KT2, 2, F], k = p+128t+256kk
    xt8 = np.ascontiguousarray(
        xr.reshape(B, 128, KT2, 2, F)
    ).astype(E4M3)
    # wt[p, k, o] = W[o, 128k+p]
    wt = np.ascontiguousarray(W.reshape(OUT, KT, 128).transpose(2, 1, 0)).astype(BF16)
    # vs8[bb, p, kk, t, r] = v[bb, 0, 128(2kk+t)+p, r]  (unscaled, e4m3)
    vs8 = np.ascontiguousarray(
        v[:, 0].reshape(B, KT2, 2, 128, R).transpose(0, 3, 1, 2, 4)
    ).astype(E4M3)
    # ut[bb, r, o] = u[bb, 0, o, r]
    ut = np.ascontiguousarray(u[:, 0].transpose(0, 2, 1)).astype(BF16)
    # biasv[c][p, bb*OT+ot] = fc_bias[128*ot+p] + b[4c+bb, 0, 128*ot+p]
    bias_full = (fc_bias[None, :] + b[:, 0]).astype(np.float32)  # [B, OUT]
    biasv = np.ascontiguousarray(
        bias_full.reshape(B, OT, 128).transpose(2, 0, 1)
    )  # [128, B, OT]

    in_maps = []
    for c in range(NCORES):
        s = slice(c * BPC, (c + 1) * BPC)
        in_maps.append(
            {
                "xt": xt[s],
                "wt": wt,
                "xt8": xt8[s],
                "vs8": vs8[s],
                "ut": ut[s],
                "biasv": np.ascontiguousarray(biasv[:, s, :]).reshape(128, BPC * OT),
            }
        )
    return in_maps


def _run(in_maps, trace=False, reps=1, **kw):
    from concourse import bass_utils

    key = reps
    if key not in _COMPILED:
        _COMPILED[key] = _build_nc(reps)
    nc = _COMPILED[key]
    res = bass_utils.run_bass_kernel_spmd(
        nc, in_maps, list(range(NCORES)), trace=trace, **kw
    )
    return res


def kernel(x, u, v, b, W, fc_bias):
    x = np.asarray(x, dtype=np.float32)
    u = np.asarray(u, dtype=np.float32)
    v = np.asarray(v, dtype=np.float32)
    b = np.asarray(b, dtype=np.float32)
    W = np.asarray(W, dtype=np.float32)
    fc_bias = np.asarray(fc_bias, dtype=np.float32)

    in_maps = _shard_inputs(x, u, v, b, W, fc_bias)
    res = _run(in_maps, trace=False)
    outs = []
    for r in res.results:
        yt = np.asarray(r["y"], dtype=np.float32)  # [BPC, OT, 128, F]
        outs.append(yt.transpose(0, 3, 1, 2).reshape(BPC, F, OUT))
    return np.concatenate(outs, axis=0)
